# revision 42
# baseline (speedup 1.0000x reference)
"""MoE-routed autoencoder (4 experts, 1024->512->128->512->1024) on 8 TRN2 cores.

Strategy (expert-parallel):
- Host: 2 cores per expert (8 cores / 4 experts). Each core gets HALF of one
  expert's atoms and ONLY that expert's weights (2.36 MB instead of 9.4 MB of
  replicated weights per core), so the head-critical DMA bytes are minimal and
  no mid-stream weight loads ever compete with the x-tile pipeline.
- Device: activations live in transposed layout [feat, atoms]; every layer is
  out[M=feat_tile, N=atom_cols] = W[K,M].T @ act[K,N] on the tensor engine
  (f16 operands, fp32 PSUM accumulation). PSUM->SBUF evictions (bias-free
  ReLU / copy) alternate between the scalar and vector engines so neither
  becomes the serial bottleneck. 48 dependency-free warmup matmuls release
  the HAM clock gate (1.2 -> 2.4 GHz) during the DMA prologue, and tile 0's
  L1 loops k-OUTER so each arriving w1/x0 chunk pair immediately feeds 4
  matmuls (real progress at DMA pace through the bandwidth-bound head ramp).
- DMA: head bytes ride both HWDGE rings in L1's k-order (w1/x0 chunk pairs
  alternating sync/scalar, w2 + x1-half on scalar) while w3/w4-halves ride
  the gpsimd SWDGE queue; body x tiles stream whole on sync. Y is stored as
  f16 (half the out-DMA of fp32) on gpsimd; the last tiles store per m-chunk
  alternating sync/gpsimd, and the final m-chunk splits its eviction + store
  across both engines/queues to shorten the drain.
- Host: scatter the routed f16 outputs back to the original atom order (fp32).
"""

import math

import numpy as np

N_CORES = 8

_PROGRAM_CACHE: dict = {}

# test-harness knobs: when _TRACE is set, the SPMD launch requests an NTFF
# profile and the BassKernelResults lands in _LAST["res"].
_TRACE = False
_LAST: dict = {}


def _build_program(dims, tiles, use_bias):
    import concourse.bass as bass  # noqa: F401
    import concourse.tile as tile
    from concourse import bacc, mybir

    D_IN, H1, LAT, D_OUT = dims
    f32 = mybir.dt.float32
    f16 = mybir.dt.float16
    RELU = mybir.ActivationFunctionType.Relu
    IDENT = mybir.ActivationFunctionType.Identity
    COPY = mybir.ActivationFunctionType.Copy

    KC1 = D_IN // 128   # 8  k-chunks layer 1
    MC1 = H1 // 128     # 4
    KC2 = H1 // 128     # 4
    MC2 = LAT // 128    # 1
    KC3 = LAT // 128    # 1
    MC3 = H1 // 128     # 4
    KC4 = H1 // 128     # 4
    MC4 = D_OUT // 128  # 8

    nt = len(tiles)
    TM = max(tiles)
    xoffs = [0]
    yoffs = [0]
    for t in tiles:
        xoffs.append(xoffs[-1] + 128 * KC1 * t)
        yoffs.append(yoffs[-1] + 128 * MC4 * t)
    n_bias_cols = (H1 + LAT + H1 + D_OUT) // 128  # 17

    nc = bacc.Bacc("TRN2", target_bir_lowering=False, debug=False,
                   num_devices=N_CORES)
    xt = nc.dram_tensor("xt", [xoffs[-1]], f16,
                        kind="ExternalInput").ap()
    w1 = nc.dram_tensor("w1", [128, KC1 * H1], f16, kind="ExternalInput").ap()
    w2 = nc.dram_tensor("w2", [128, KC2 * LAT], f16, kind="ExternalInput").ap()
    w3 = nc.dram_tensor("w3", [128, KC3 * H1], f16, kind="ExternalInput").ap()
    w4 = nc.dram_tensor("w4", [128, KC4 * D_OUT], f16,
                        kind="ExternalInput").ap()
    if use_bias:
        bias = nc.dram_tensor("bias", [128, n_bias_cols], f32,
                              kind="ExternalInput").ap()
    yt = nc.dram_tensor("yt", [yoffs[-1]], f16,
                        kind="ExternalOutput").ap()

    with tile.TileContext(nc) as tc:
        with (
            tc.tile_pool(name="wp", bufs=1) as wp,
            tc.tile_pool(name="xp", bufs=min(nt, 10)) as xp,
            tc.tile_pool(name="hp", bufs=3) as hp,
            tc.tile_pool(name="zp", bufs=3) as zp,
            tc.tile_pool(name="dp", bufs=3) as dp,
            tc.tile_pool(name="yp", bufs=3) as yp,
            tc.tile_pool(name="bp", bufs=1) as bp,
            tc.tile_pool(name="pp1", bufs=4, space="PSUM") as pp1,
            tc.tile_pool(name="pp23", bufs=2, space="PSUM") as pp23,
            tc.tile_pool(name="ppb", bufs=2, space="PSUM") as ppb,
        ):
            if use_bias:
                btile = bp.tile([128, n_bias_cols], f32)

            # evictions alternate scalar/vector so neither engine serializes
            # the PSUM drain. With bias they all go to scalar (has bias port).
            ec = [0]

            def evict(out_ap, ps_ap, relu, bcol):
                if use_bias:
                    b = btile[:, bcol:bcol + 1]
                    nc.scalar.activation(out_ap, ps_ap,
                                         RELU if relu else IDENT, bias=b)
                    return
                on_scalar = (ec[0] % 2 == 0)
                ec[0] += 1
                if on_scalar:
                    nc.scalar.activation(out_ap, ps_ap,
                                         RELU if relu else COPY)
                elif relu:
                    nc.vector.tensor_scalar_max(out_ap, ps_ap, 0.0)
                else:
                    nc.vector.tensor_copy(out_ap, ps_ap)

            # PE warmup: dependency-free matmuls on a memset tile release the
            # HAM clock gate (1.2 -> 2.4 GHz) while the DMA prologue runs, so
            # the first real tiles are not clock-throttled. The memset rides
            # gpsimd (its queue starts right after the start barrier) so the
            # warmup isn't delayed behind the vector engine's table loads.
            warm = bp.tile([128, 128], f16, tag="warm")
            nc.gpsimd.memset(warm[:], 0.0)
            wps = ppb.tile([128, 128], f32, tag="ps")
            for _ in range(48):
                nc.tensor.matmul(wps[:], warm[:], warm[:],
                                 start=True, stop=True)

            w1t = wp.tile([128, KC1 * H1], f16, tag="w1")
            w2t = wp.tile([128, KC2 * LAT], f16, tag="w2")
            w3t = wp.tile([128, KC3 * H1], f16, tag="w3")
            w4t = wp.tile([128, KC4 * D_OUT], f16, tag="w4")

            xtiles = {}
            for si in range(nt):
                T = tiles[si]
                xo = xoffs[si]
                xap = xt[xo:xo + 128 * KC1 * T].rearrange("(p f) -> p f",
                                                          p=128)
                if si in xtiles:
                    xtile = xtiles.pop(si)
                elif si == 0:
                    xtile = xp.tile([128, KC1 * TM], f16, tag="x")
                    # w3/w4-halves (+bias) ride the otherwise-idle gpsimd
                    # queue; w4 is m-chunk-major so L4 can start on the first
                    # half.
                    nc.gpsimd.dma_start(w3t[:], w3[:])
                    half4 = (MC4 // 2) * H1
                    nc.gpsimd.dma_start(w4t[:, :half4], w4[:, :half4])
                    nc.gpsimd.dma_start(w4t[:, half4:], w4[:, half4:])
                    if use_bias:
                        nc.gpsimd.dma_start(btile[:], bias[:])
                    # head critical path: the sync HWDGE ring is ~2-4x faster
                    # than the scalar ring (measured 240-425 vs ~110 GB/s),
                    # so the bulk (k01/k23/k45 pairs + x1's first half) rides
                    # sync while the small w2 + the k67 pair + x1's second
                    # half ride scalar in parallel, all in L1's k-order.
                    nc.scalar.dma_start(w2t[:], w2[:])
                    for k in range(0, KC1, 2):
                        q = nc.sync if (k // 2) % 2 == 0 else nc.scalar
                        q.dma_start(w1t[:, k * H1:(k + 2) * H1],
                                    w1[:, k * H1:(k + 2) * H1])
                        q.dma_start(xtile[:, k * T:(k + 2) * T],
                                    xap[:, k * T:(k + 2) * T])
                    if nt > 1:
                        T1 = tiles[1]
                        xn = xp.tile([128, KC1 * TM], f16, tag="x")
                        x1ap = xt[xoffs[1]:xoffs[1] + 128 * KC1 * T1]\
                            .rearrange("(p f) -> p f", p=128)
                        half = (KC1 // 2) * T1
                        nc.sync.dma_start(xn[:, :half], x1ap[:, :half])
                        nc.scalar.dma_start(xn[:, half:KC1 * T1],
                                            x1ap[:, half:])
                        xtiles[1] = xn
                else:
                    xtile = xp.tile([128, KC1 * TM], f16, tag="x")
                    nc.sync.dma_start(xtile[:, :KC1 * T], xap)

                # L1: h[H1, T] = relu(W1.T @ x)
                htile = hp.tile([128, MC1 * TM], f16, tag="h")
                if si == 0:
                    # tile 0 runs DMA-paced: loop k OUTER so each arriving
                    # w1/x0 chunk immediately feeds 4 matmuls (one per m-bank)
                    # — real progress at DMA pace instead of idling in m=0's
                    # k-loop while later chunks stream in.
                    pss = [pp1.tile([128, T], f32, tag="ps", name=f"ps0_{m}")
                           for m in range(MC1)]
                    for k in range(KC1):
                        for m in range(MC1):
                            nc.tensor.matmul(
                                pss[m][:],
                                w1t[:, k * H1 + m * 128:k * H1 + (m + 1) * 128],
                                xtile[:, k * T:(k + 1) * T],
                                start=(k == 0), stop=(k == KC1 - 1))
                    for m in range(MC1):
                        evict(htile[:, m * T:(m + 1) * T], pss[m][:], True, m)
                else:
                    for m in range(MC1):
                        ps = pp1.tile([128, T], f32, tag="ps")
                        for k in range(KC1):
                            nc.tensor.matmul(
                                ps[:],
                                w1t[:, k * H1 + m * 128:k * H1 + (m + 1) * 128],
                                xtile[:, k * T:(k + 1) * T],
                                start=(k == 0), stop=(k == KC1 - 1))
                        evict(htile[:, m * T:(m + 1) * T], ps[:], True, m)

                # L2: z[LAT, T] = relu(W2.T @ h)
                ztile = zp.tile([128, MC2 * TM], f16, tag="z")
                for m in range(MC2):
                    ps = pp23.tile([128, T], f32, tag="ps")
                    for k in range(KC2):
                        nc.tensor.matmul(
                            ps[:],
                            w2t[:, k * LAT + m * 128:k * LAT + (m + 1) * 128],
                            htile[:, k * T:(k + 1) * T],
                            start=(k == 0), stop=(k == KC2 - 1))
                    evict(ztile[:, m * T:(m + 1) * T], ps[:], True, MC1 + m)

                # L3: d[H1, T] = relu(W3.T @ z)
                dtile = dp.tile([128, MC3 * TM], f16, tag="d")
                for m in range(MC3):
                    ps = pp23.tile([128, T], f32, tag="ps")
                    for k in range(KC3):
                        nc.tensor.matmul(
                            ps[:],
                            w3t[:, k * H1 + m * 128:k * H1 + (m + 1) * 128],
                            ztile[:, k * T:(k + 1) * T],
                            start=(k == 0), stop=(k == KC3 - 1))
                    evict(dtile[:, m * T:(m + 1) * T], ps[:], True,
                          MC1 + MC2 + m)

                # L4: y[D_OUT, T] = W4.T @ d  (no relu), f16 out
                yo = yoffs[si]
                yap = yt[yo:yo + 128 * MC4 * T].rearrange("(p f) -> p f",
                                                          p=128)
                ytile = yp.tile([128, MC4 * TM], f16, tag="y")
                for m in range(MC4):
                    ps = ppb.tile([128, T], f32, tag="ps")
                    for k in range(KC4):
                        # w4 is m-chunk-major: col = m*H1 + k*128
                        nc.tensor.matmul(
                            ps[:],
                            w4t[:, m * H1 + k * 128:m * H1 + (k + 1) * 128],
                            dtile[:, k * T:(k + 1) * T],
                            start=(k == 0), stop=(k == KC4 - 1))
                    if si == nt - 1 and m == MC4 - 1 and not use_bias:
                        # very last chunk: split the eviction + store in half
                        # across both engines/queues so the final drain chain
                        # is as short as possible
                        h = (T // 16) * 8
                        lo = ytile[:, m * T:m * T + h]
                        hi = ytile[:, m * T + h:(m + 1) * T]
                        nc.scalar.activation(lo, ps[:, :h], COPY)
                        nc.vector.tensor_copy(hi, ps[:, h:])
                        nc.sync.dma_start(yap[:, m * T:m * T + h], lo)
                        nc.gpsimd.dma_start(yap[:, m * T + h:(m + 1) * T], hi)
                        continue
                    evict(ytile[:, m * T:(m + 1) * T], ps[:], False,
                          MC1 + MC2 + MC3 + m)
                    if si >= nt - 2:
                        # last tiles: store per m-chunk, alternating queues,
                        # so the final DMA drain is small and overlapped
                        q = nc.sync if (si == nt - 1 and m % 2 == 1) \
                            else nc.gpsimd
                        q.dma_start(yap[:, m * T:(m + 1) * T],
                                    ytile[:, m * T:(m + 1) * T])
                if si < nt - 2:
                    nc.gpsimd.dma_start(yap, ytile[:, :MC4 * T])

    nc.compile()
    return nc


def kernel(**inputs) -> np.ndarray:
    from concourse.bass_utils import run_bass_kernel_spmd

    X = np.ascontiguousarray(inputs["X"], dtype=np.float32)
    sym_ids = np.asarray(inputs["sym_ids"]).astype(np.int64).ravel()
    We = [inputs["We1"], inputs["We2"], inputs["Wd1"], inputs["Wd2"]]
    be = [np.asarray(inputs["be1"], dtype=np.float32),
          np.asarray(inputs["be2"], dtype=np.float32),
          np.asarray(inputs["bd1"], dtype=np.float32),
          np.asarray(inputs["bd2"], dtype=np.float32)]

    N, D_IN = X.shape
    E, _, H1 = np.asarray(We[0]).shape
    LAT = np.asarray(We[1]).shape[2]
    D_OUT = np.asarray(We[3]).shape[2]
    KC1 = D_IN // 128
    MC4 = D_OUT // 128
    use_bias = any(np.any(b) for b in be)

    # ---- host routing: expert-parallel, N_CORES//E cores per expert ----
    G = N_CORES // E
    idx_e = [np.flatnonzero(sym_ids == e) for e in range(E)]
    Cmax = max(1, max(-(-len(ix) // G) for ix in idx_e))
    # uniform column tiles of width <=512 (PSUM bank limit), multiple of 8.
    # (Narrower head/tail tile variants measured neutral-to-worse, and a
    # narrow FIRST tile trips an unresolved device-side failure — widths
    # must never grow tile-over-tile.)
    nt = max(1, math.ceil(Cmax / 512))
    T = -(-math.ceil(Cmax / nt) // 8) * 8
    tiles = [T] * nt
    C_tot = sum(tiles)

    # ---- build / fetch compiled program ----
    dims = (D_IN, H1, LAT, D_OUT)
    key = (dims, tuple(tiles), use_bias)
    nc = _PROGRAM_CACHE.get(key)
    if nc is None:
        nc = _build_program(dims, tiles, use_bias)
        _PROGRAM_CACHE[key] = nc

    # ---- prepare inputs ----
    rnd = lambda a: np.ascontiguousarray(np.asarray(a, np.float32)).astype(
        np.float16)
    XrT = np.ascontiguousarray(rnd(X).T)                     # [D_IN, N]
    XrT_z = np.concatenate(
        [XrT, np.zeros((D_IN, 1), np.float16)], axis=1)      # pad col = N

    # weights in device layout: [128, kc*mw] (k-chunk-major columns)
    def wdev(w, kc, mw):
        return np.ascontiguousarray(
            rnd(w).reshape(kc, 128, mw).transpose(1, 0, 2).reshape(128, kc * mw))

    # w4 in m-chunk-major layout: col = m*H1 + k*128 + j
    def wdev4(w):
        kc, mc = H1 // 128, D_OUT // 128
        return np.ascontiguousarray(
            rnd(w).reshape(kc, 128, mc, 128).transpose(1, 2, 0, 3)
            .reshape(128, mc * kc * 128))

    n_bias_cols = (H1 + LAT + H1 + D_OUT) // 128

    bounds = np.cumsum([0] + tiles)
    perms = []
    in_maps = []
    for c in range(N_CORES):
        e = c // G
        part = np.array_split(idx_e[e], G)[c % G]
        perm = np.full(C_tot, N, dtype=np.int64)
        perm[:len(part)] = part
        perms.append(perm)
        g3 = XrT_z[:, perm].reshape(KC1, 128, C_tot)
        xflat = np.concatenate(
            [np.ascontiguousarray(
                g3[:, :, bounds[t]:bounds[t + 1]].transpose(1, 0, 2)).reshape(-1)
             for t in range(nt)])
        m = {"xt": xflat,
             "w1": wdev(We[0][e], KC1, H1),
             "w2": wdev(We[1][e], H1 // 128, LAT),
             "w3": wdev(We[2][e], LAT // 128, H1),
             "w4": wdev4(We[3][e])}
        if use_bias:
            bias_h = np.zeros((128, n_bias_cols), np.float32)
            col = 0
            for b in (be[0][e], be[1][e], be[2][e], be[3][e]):
                for mch in range(len(b) // 128):
                    bias_h[:, col] = b[mch * 128:(mch + 1) * 128]
                    col += 1
            m["bias"] = bias_h
        in_maps.append(m)

    res = run_bass_kernel_spmd(nc, in_maps, core_ids=list(range(N_CORES)),
                               trace=_TRACE)
    _LAST["res"] = res

    # ---- unshard ----
    Y = np.empty((N, D_OUT), dtype=np.float32)
    for c in range(N_CORES):
        yflat = np.asarray(res.results[c]["yt"])
        ytc = np.empty((D_OUT, C_tot), dtype=np.float16)
        yo = 0
        for t in range(nt):
            T = tiles[t]
            ytc[:, bounds[t]:bounds[t + 1]] = (
                yflat[yo:yo + 128 * MC4 * T].reshape(128, MC4, T)
                .transpose(1, 0, 2).reshape(D_OUT, T))
            yo += 128 * MC4 * T
        perm = perms[c]
        valid = perm != N
        Y[perm[valid]] = ytc.T[valid].astype(np.float32)
    return Y
